# revision 1
# baseline (speedup 1.0000x reference)
"""MinLSTM cell kernel for 8x Trainium2 NeuronCores.

Strategy: data-parallel over batch (B=256 -> 32 rows/core). Everything on
device lives in a "u-on-partitions" layout so no on-device transposes are
needed (the host does all layout work for free):

  - host pre-transposes x to [d, t, b] per core, so the fused input
    projection xw = x @ [Wf|Wi|Wc] runs as out[n, (t,b)] with W stationary
    and x^T moving (fp32r, full PE rate at N=512).
  - per-partition gate bias (b_cat - colsum(U)) is folded into the
    PSUM->SBUF eviction via tensor_scalar.
  - the recurrence uses s = sigma(2c) (so h = 2s - 1 = tanh(c)); gates are
    xw' + s @ (2U), which removes any affine fixup from the critical path.
  - scan step: an identity matmul preloads xw_t into PSUM off the critical
    path, then 12 fp32r matmuls (U2 stationary [128,128] tiles, s moving
    [128,32]) accumulate the recurrent term; sigma on the f,i columns and
    tanh on the cc columns read PSUM directly (ScalarE, one table set);
    3 DVE tensor_tensor ops for c = f*c + i*cc; sigma(2c) -> s; the output
    h = 2s - 1 is an off-chain DVE affine.
  - outputs stored as [u=128p, t, j, b] and re-assembled on host.
"""
import os
# The axon NTFF profile hook module is absent in this container; a stray
# BASS_TRACE=1 in the environment would crash run_bass_kernel_spmd.
os.environ["BASS_NEVER_TRACE"] = "1"

import numpy as np
import ml_dtypes
from contextlib import ExitStack

import concourse.bass as bass
import concourse.bacc as bacc
import concourse.tile as tile
import concourse.mybir as mybir
from concourse.bass_utils import run_bass_kernel_spmd

F32 = mybir.dt.float32
F32R = mybir.dt.float32r
BF16 = mybir.dt.bfloat16
AF = mybir.ActivationFunctionType
OP = mybir.AluOpType

B, T, D, U3, UN = 256, 512, 256, 768, 256
NCORES = 8
BC = B // NCORES          # 32 batch rows per core
TC = 32                   # timesteps per chunk
NCHUNK = T // TC


def _build():
    nc = bacc.Bacc("TRN2", target_bir_lowering=False, debug=False)

    xt = nc.declare_dram_parameter("xt", [D, T, BC], F32R, isOutput=False)
    wt = nc.declare_dram_parameter("wt", [D, U3], F32R, isOutput=False)
    uh = nc.declare_dram_parameter("uh", [D, U3], F32R, isOutput=False)
    bp = nc.declare_dram_parameter("bp", [128, 6], F32, isOutput=False)
    ident = nc.declare_dram_parameter("ident", [128, 128], F32R, isOutput=False)
    s0 = nc.declare_dram_parameter("s0", [128, 64], F32R, isOutput=False)
    c0 = nc.declare_dram_parameter("c0", [128, 64], F32, isOutput=False)
    hout = nc.declare_dram_parameter("hout", [128, T * 64], F32, isOutput=True)

    with tile.TileContext(nc) as tc, ExitStack() as ctx:
        const = ctx.enter_context(tc.tile_pool(name="const", bufs=1))
        xt_pool = ctx.enter_context(tc.tile_pool(name="xt", bufs=2))
        xw_pool = ctx.enter_context(tc.tile_pool(name="xw", bufs=2))
        ho_pool = ctx.enter_context(tc.tile_pool(name="ho", bufs=2))
        work = ctx.enter_context(tc.tile_pool(name="work", bufs=3))
        ps_g = ctx.enter_context(tc.tile_pool(name="psg", bufs=2, space="PSUM"))
        ps_s = ctx.enter_context(tc.tile_pool(name="pss", bufs=2, space="PSUM"))

        # constants / persistent state
        w_sb = const.tile([128, 2 * U3], F32R)       # W tiles: [:, 768k + n]
        uh_sb = const.tile([128, 2 * U3], F32R)      # 2*U tiles, same packing
        bp_sb = const.tile([128, 6], F32)
        id_sb = const.tile([128, 128], F32R)
        s_sb = const.tile([128, 64], F32R)           # sigma(2c), col = 32j + b
        c_sb = const.tile([128, 64], F32)
        for k in range(2):
            nc.sync.dma_start(w_sb[:, k * U3:(k + 1) * U3], wt[k * 128:(k + 1) * 128, :])
            nc.sync.dma_start(uh_sb[:, k * U3:(k + 1) * U3], uh[k * 128:(k + 1) * 128, :])
        nc.sync.dma_start(bp_sb[:], bp[:])
        nc.sync.dma_start(id_sb[:], ident[:])
        nc.sync.dma_start(s_sb[:], s0[:])
        nc.sync.dma_start(c_sb[:], c0[:])

        for ch in range(NCHUNK):
            t0 = ch * TC
            # ---- load x^T chunk: two K-halves [128, TC*32] ----
            xt_t0 = xt_pool.tile([128, TC * BC], F32R, tag="xt0")
            xt_t1 = xt_pool.tile([128, TC * BC], F32R, tag="xt1")
            nc.sync.dma_start(xt_t0[:], xt[0:128, t0:t0 + TC, :])
            nc.sync.dma_start(xt_t1[:], xt[128:256, t0:t0 + TC, :])
            xt_k = (xt_t0, xt_t1)

            # ---- xw GEMM for this chunk: out[n-tile jj, (t', b)] ----
            xw_sb = xw_pool.tile([128, TC * 192], F32R)
            xw_v = xw_sb[:].rearrange("p (t g) -> p t g", g=192)
            nhalves = (TC * BC) // 512
            for jj in range(6):
                for nh in range(nhalves):
                    psg = ps_g.tile([128, 512], F32, tag="psg")
                    for k in range(2):
                        nc.tensor.matmul(
                            psg[:],
                            w_sb[:, k * U3 + 128 * jj: k * U3 + 128 * jj + 128],
                            xt_k[k][:, nh * 512:(nh + 1) * 512],
                            start=(k == 0), stop=(k == 1),
                        )
                    # evict + per-partition bias add
                    nc.vector.tensor_scalar(
                        xw_v[:, nh * 16:(nh + 1) * 16, 32 * jj:32 * jj + 32],
                        psg[:].rearrange("p (t g) -> p t g", g=32),
                        bp_sb[:, jj:jj + 1], None, op0=OP.add,
                    )

            # ---- output staging for this chunk ----
            ho_sb = ho_pool.tile([128, TC * 64], F32)

            # ---- the sequential scan ----
            for tp in range(TC):
                # f,i gates and the cc gate go to separate PSUM banks so the
                # cc tanh overlaps the f,i matmul block instead of waiting
                # for all 12 recurrent matmuls.
                psfi = ps_s.tile([128, 128], F32, tag="psfi")
                pscc = ps_s.tile([128, 64], F32, tag="pscc")
                nc.tensor.matmul(psfi[:], id_sb[:], xw_v[:, tp, 0:128],
                                 start=True, stop=False, skip_group_check=True)
                nc.tensor.matmul(pscc[:], id_sb[:], xw_v[:, tp, 128:192],
                                 start=True, stop=False, skip_group_check=True)
                for jj in range(4):
                    for k in range(2):
                        nc.tensor.matmul(
                            psfi[:, 32 * jj:32 * jj + 32],
                            uh_sb[:, k * U3 + 128 * jj: k * U3 + 128 * jj + 128],
                            s_sb[:, 32 * k:32 * k + 32],
                            start=False, stop=(jj == 3 and k == 1),
                            skip_group_check=True,
                        )
                fi = work.tile([128, 128], F32, tag="fi")
                nc.scalar.activation(fi[:], psfi[:], AF.Sigmoid)
                for jj in range(4, 6):
                    for k in range(2):
                        nc.tensor.matmul(
                            pscc[:, 32 * (jj - 4):32 * (jj - 4) + 32],
                            uh_sb[:, k * U3 + 128 * jj: k * U3 + 128 * jj + 128],
                            s_sb[:, 32 * k:32 * k + 32],
                            start=False, stop=(jj == 5 and k == 1),
                            skip_group_check=True,
                        )
                cc = work.tile([128, 64], F32, tag="cc")
                nc.scalar.activation(cc[:], pscc[:], AF.Tanh)
                m1 = work.tile([128, 64], F32, tag="m1")
                nc.vector.tensor_tensor(m1[:], fi[:, 0:64], c_sb[:], op=OP.mult)
                m2 = work.tile([128, 64], F32, tag="m2")
                nc.vector.tensor_tensor(m2[:], fi[:, 64:128], cc[:], op=OP.mult)
                nc.vector.tensor_tensor(c_sb[:], m1[:], m2[:], op=OP.add)
                nc.scalar.activation(s_sb[:], c_sb[:], AF.Sigmoid, scale=2.0)
                # h = 2*s - 1 (= tanh(c)) on DVE, off the ScalarE chain
                nc.vector.tensor_scalar(
                    ho_sb[:, tp * 64:(tp + 1) * 64], s_sb[:].bitcast(F32),
                    2.0, 1.0, op0=OP.mult, op1=OP.subtract)

            nc.sync.dma_start(hout[:, t0 * 64:(t0 + TC) * 64], ho_sb[:])

    nc.compile()
    return nc


_NC_CACHE = None
_LAST_RES = None


def kernel(x, Wf, Uf, bf, Wi, Ui, bi, Wc, Uc, bc, h0, c0):
    global _NC_CACHE
    x = np.ascontiguousarray(np.asarray(x, dtype=np.float32))
    W = np.concatenate([np.asarray(Wf), np.asarray(Wi), np.asarray(Wc)], axis=1).astype(np.float32)
    Ucat = np.concatenate([np.asarray(Uf), np.asarray(Ui), np.asarray(Uc)], axis=1).astype(np.float32)
    bcat = np.concatenate([np.asarray(bf), np.asarray(bi), np.asarray(bc)]).astype(np.float32)
    h0 = np.asarray(h0, dtype=np.float32)
    c0 = np.asarray(c0, dtype=np.float32)

    Uh2 = 2.0 * Ucat                                  # s @ (2U) with s = (h+1)/2
    bias = bcat - Ucat.sum(axis=0)                    # absorbs the "-1" of h = 2s-1
    bp2 = np.empty((128, 6), np.float32)
    for jj in range(6):
        bp2[:, jj] = bias[128 * jj:128 * (jj + 1)]

    if _NC_CACHE is None:
        _NC_CACHE = _build()
    nc = _NC_CACHE

    in_maps = []
    for r in range(NCORES):
        xs = x[r * BC:(r + 1) * BC]                   # [32, T, D]
        xtr = np.ascontiguousarray(xs.transpose(2, 1, 0))   # [D, T, 32]
        h0s = h0[r * BC:(r + 1) * BC]                 # [32, 256]
        c0s = c0[r * BC:(r + 1) * BC]
        # [128, 64] with col = 32j + b, partition p -> u = 128j + p
        s0t = np.empty((128, 64), np.float32)
        c0t = np.empty((128, 64), np.float32)
        for j in range(2):
            s0t[:, 32 * j:32 * (j + 1)] = (h0s[:, 128 * j:128 * (j + 1)].T + 1.0) / 2.0
            c0t[:, 32 * j:32 * (j + 1)] = c0s[:, 128 * j:128 * (j + 1)].T
        in_maps.append({"xt": xtr, "wt": W, "uh": Uh2, "bp": bp2,
                        "ident": np.eye(128, dtype=np.float32),
                        "s0": s0t, "c0": c0t})

    res = run_bass_kernel_spmd(nc, in_maps, list(range(NCORES)))
    global _LAST_RES
    _LAST_RES = res

    out = np.empty((B, T, UN), np.float32)
    for r in range(NCORES):
        ho = res.results[r]["hout"].reshape(128, T, 2, BC)
        # [p, t, j, b] -> [b, t, j*128 + p]
        out[r * BC:(r + 1) * BC] = ho.transpose(3, 1, 2, 0).reshape(BC, T, UN)
    return out

